# revision 1
# baseline (speedup 1.0000x reference)
"""Trainium2 Bass kernel for a single-layer MHA + FFN transformer block.

Reference computation (for x: [1, 4096, 768], 12 heads, dff=3072):
    qkv = (x @ w_qkv + b_qkv)  -> q, k, v       [t, 768] each
    scores = q k^T / sqrt(768) ; wei = softmax(scores)
    attn = wei @ v  (concat heads)              [t, 768]
    h = gelu(attn @ w_ff1 + b_ff1)              [t, 3072]
    out = h @ w_ff2 + b_ff2                     [t, 768]

Sharding: sequence-parallel over the 4096 tokens across 8 NeuronCores
(512 rows each). Every core computes q/k/v for its own rows, the k/v
blocks are exchanged with one bf16 AllGather, then each core runs full
attention for its 512 query rows over all 4096 keys plus the whole FFN
for its rows. Outputs are concatenated on the host.

Host-side prep (all cheap numpy): x is pre-transposed per core (bf16 for
the q/k path, f32 for the v path), w_qkv's q/k columns are pre-cast to
bf16 and laid out d-tile-major so every weight load is a contiguous DMA,
and w_ff1 is laid out so the whole matrix loads with one DMA.

The v tiles carry a per-head ones column through the AllGather (width
65*12=780) so softmax denominators ride along in the wei@v matmuls and
the gathered v reads back with fully contiguous DMAs. All 8 ranks' k/v
(including our own) are read back from the gather output so the program
stays rank-uniform.

Precision: q/k projections run bf16 (scores are bf16 anyway), v and the
FFN run fp32r (full PE rate at free-dim>=256); softmax skips
max-subtraction because the logits here are bounded by ~0.6.
"""

import json as _json
import math

import numpy as np
import ml_dtypes

import concourse.bass as bass
import concourse.mybir as mybir
import concourse.tile as tile
from concourse.bass_utils import run_bass_kernel_spmd

# ---------------------------------------------------------------------------
# Workaround: the pinned walrus build only supports ONE embedded semaphore
# wait per instruction, but Tile's sem assigner attaches several. Split the
# excess onto standalone EventSemaphore instructions (pure waits) inserted
# just before the over-subscribed instruction (same engine => same program
# order, identical semantics).
# ---------------------------------------------------------------------------
_MAX_WAITS = 1
_ctr = [0]
if not getattr(bass.Bass, "_multiwait_patched", False):
    _orig_to_json_bytes = bass.Bass.to_json_bytes

    def _split_multiwait_json_bytes(self):
        bir = _json.loads(_orig_to_json_bytes(self))
        for f in bir["functions"]:
            for b in f["blocks"]:
                new_insts = []
                for inst in b["instructions"]:
                    si = inst.get("sync_info")
                    waits = si.get("on_wait", []) if si else []
                    if len(waits) > _MAX_WAITS:
                        excess, keep = waits[:-_MAX_WAITS], waits[-_MAX_WAITS:]
                        for k in range(0, len(excess), _MAX_WAITS):
                            _ctr[0] += 1
                            new_insts.append({
                                "debug": inst.get("debug", 0),
                                "engine": inst["engine"],
                                "ins": [], "outs": [],
                                "name": "I-waitsplit-%d" % _ctr[0],
                                "opcode": "EventSemaphore",
                                "sync_info": {"on_update": [],
                                              "on_wait": excess[k:k + _MAX_WAITS]},
                            })
                        si["on_wait"] = keep
                    new_insts.append(inst)
                b["instructions"] = new_insts
        return _json.dumps(bir).encode()

    bass.Bass.to_json_bytes = _split_multiwait_json_bytes
    bass.Bass._multiwait_patched = True

F32 = mybir.dt.float32
F32R = mybir.dt.float32r
BF16 = mybir.dt.bfloat16
AFT = mybir.ActivationFunctionType

R = 8          # cores
T = 4096       # sequence length
TL = T // R    # rows per core (512)
D = 768
H = 12
HD = D // H    # 64
DFF = 4 * D    # 3072
P = 128
NDT = D // P   # 6 d-tiles
NTT = TL // P  # 4 local t-tiles
NFT = DFF // P  # 24 dff tiles
NCH = T // P   # 32 global key chunks
SCALE = 1.0 / math.sqrt(D)
VW = H * (HD + 1)      # 780: v tile width with a ones column per head
KV2 = D * TL + TL * VW  # 792576 elems in the AllGather payload per rank

_NC_CACHE = {}


def _build_nc():
    nc = bass.Bass(num_devices=R)
    # host-prepped per-core inputs
    xTb = nc.declare_dram_parameter("xTb", [P, NDT, TL], BF16, isOutput=False)
    # host-prepped common weights
    wqk = nc.declare_dram_parameter("wqk", [P, NDT, 2 * D], BF16, isOutput=False)
    wv = nc.declare_dram_parameter("wv", [P, NDT, D], BF16, isOutput=False)
    w1h = nc.declare_dram_parameter("w1h", [P, NDT, DFF], BF16, isOutput=False)
    w2b = nc.declare_dram_parameter("w2b", [P, NFT, D], BF16, isOutput=False)
    b_qkv = nc.declare_dram_parameter("b_qkv", [3 * D], F32, isOutput=False)
    b_ff1 = nc.declare_dram_parameter("b_ff1", [DFF], F32, isOutput=False)
    b_ff2 = nc.declare_dram_parameter("b_ff2", [D], F32, isOutput=False)
    y = nc.declare_dram_parameter("y", [TL, D], F32, isOutput=True)

    from contextlib import ExitStack

    with tile.TileContext(nc) as tc, ExitStack() as top:
        const = top.enter_context(tc.tile_pool(name="const", bufs=1))
        dramp = top.enter_context(tc.tile_pool(name="dramp", bufs=1, space="DRAM"))
        persist = top.enter_context(tc.tile_pool(name="persist", bufs=1))

        ones_dram = nc.inline_tensor(np.ones((1, P), np.float32), name="ones_const")
        ones_row = const.tile([1, P], F32R, name="ones_row")
        nc.sync.dma_start(ones_row[:], ones_dram.ap().bitcast(F32R))
        # head-pair selection for the denominator broadcast: row 0 -> cols
        # 0..63 (even head), row 1 -> cols 64..127 (odd head)
        sel_np = np.zeros((2, P), np.float32)
        sel_np[0, 0:HD] = 1.0
        sel_np[1, HD:P] = 1.0
        sel_dram = nc.inline_tensor(sel_np, name="sel2_const")
        sel2 = const.tile([2, P], F32R, name="sel2")
        nc.sync.dma_start(sel2[:], sel_dram.ap().bitcast(F32R))

        bq_sb = const.tile([P, 2 * NDT], F32, name="bq_sb")
        nc.gpsimd.dma_start(
            bq_sb[:], b_qkv.ap()[0:2 * D].rearrange("(o p) -> p o", p=P))
        bv_sb = const.tile([1, D], F32R, name="bv_sb")
        nc.gpsimd.dma_start(bv_sb[:], b_qkv.ap()[None, 2 * D:3 * D].bitcast(F32R))
        b1_sb = const.tile([P, NFT], F32, name="b1_sb")
        nc.gpsimd.dma_start(b1_sb[:], b_ff1.ap().rearrange("(o p) -> p o", p=P))
        b2_sb = const.tile([1, D], F32R, name="b2_sb")
        nc.gpsimd.dma_start(b2_sb[:], b_ff2.ap()[None, :].bitcast(F32R))

        # preload the exp activation table while phase 1 is DMA-bound
        warmup = const.tile([P, 1], F32, name="warmup")
        nc.scalar.activation(warmup[:], bq_sb[:, 0:1], AFT.Exp)

        ag_in = dramp.tile([KV2], BF16, name="ag_in")
        ag_out = dramp.tile([R * KV2], BF16, addr_space="Shared", name="ag_out")

        attnT = [persist.tile([P, TL], BF16, name=f"attnT{i}") for i in range(NDT)]
        w1sb = persist.tile([P, NDT, DFF], BF16, name="w1sb")
        w2sb = persist.tile([P, NFT, D], BF16, name="w2sb")

        kv_scope = top.enter_context(ExitStack())
        kvp = kv_scope.enter_context(tc.tile_pool(name="kvp", bufs=1))
        qT = [kvp.tile([P, TL], BF16, name=f"qT{i}") for i in range(NDT)]
        vf = [kvp.tile([P, VW], BF16, name=f"vf{c}") for c in range(NCH)]

        # ------------------------------------------------------------------
        # Phase 1: QKV projections straight from host-transposed x
        # ------------------------------------------------------------------
        with ExitStack() as ph1:
            xp = ph1.enter_context(tc.tile_pool(name="xp", bufs=1))
            psQ = ph1.enter_context(tc.tile_pool(name="psQ", bufs=2, space="PSUM"))
            psV = ph1.enter_context(tc.tile_pool(name="psV", bufs=2, space="PSUM"))

            xtb_sb = xp.tile([P, NDT, TL], BF16, name="xtb")
            nc.sync.dma_start(xtb_sb[:], xTb.ap())
            wqk_sb = xp.tile([P, NDT, 2 * D], BF16, name="wqk_sb")
            nc.sync.dma_start(wqk_sb[:], wqk.ap())
            wv_sb = xp.tile([P, NDT, D], BF16, name="wv_sb")
            nc.scalar.dma_start(wv_sb[:], wv.ap())

            def proj_jt(jt, out_ap):
                """qkv^T tile for q/k channel block jt (0..11)."""
                ps = psQ.tile([P, TL], F32, tag="psq", name="psq")
                for d_ in range(NDT):
                    nc.tensor.matmul(ps[:], wqk_sb[:, d_, P * jt:P * (jt + 1)],
                                     xtb_sb[:, d_, :],
                                     start=(d_ == 0), stop=(d_ == NDT - 1))
                nc.vector.tensor_scalar_add(out_ap, ps[:], bq_sb[:, jt:jt + 1])

            # k first (AllGather input): k channel blocks are jt 6..11.
            # Stage the local k/v blocks in vf tiles — the gather readback
            # overwrites them later (Tile serializes write-after-read).
            ag_k = ag_in[0:D * TL].rearrange("(a b) -> a b", b=TL)
            kT_loc = [vf[i][:, 0:TL] for i in range(NDT)]
            for i in range(NDT):
                proj_jt(NDT + i, kT_loc[i])
                nc.sync.dma_start(ag_k[P * i:P * (i + 1), :], kT_loc[i])

            # v in [t, j] orientation with the per-head ones columns baked in
            ag_v = ag_in[D * TL:].rearrange("(a b) -> a b", b=VW)
            v_loc = [vf[NDT + t] for t in range(NTT)]
            for tt in range(NTT):
                vfv = v_loc[tt].rearrange("p (h e) -> p h e", e=HD + 1)
                nc.vector.memset(vfv[:, :, HD:HD + 1], 1.0)
                for o2 in range(2):
                    sl = slice(384 * o2, 384 * (o2 + 1))
                    ps = psV.tile([P, 384], F32, tag="psv", name="psv")
                    for d_ in range(NDT):
                        nc.tensor.matmul(ps[:],
                                         xtb_sb[:, d_, P * tt:P * (tt + 1)],
                                         wv_sb[:, d_, sl],
                                         start=(d_ == 0), stop=False)
                    nc.tensor.matmul(ps[:], ones_row[:], bv_sb[:, sl],
                                     start=False, stop=True)
                    dst = vfv[:, 6 * o2:6 * (o2 + 1), 0:HD]
                    nc.vector.tensor_copy(
                        dst, ps[:].rearrange("p (h e) -> p h e", e=HD))
                nc.sync.dma_start(ag_v[P * tt:P * (tt + 1), :], v_loc[tt][:])

            nc.gpsimd.collective_compute(
                "AllGather", mybir.AluOpType.bypass,
                replica_groups=[list(range(R))],
                ins=[ag_in[:]], outs=[ag_out[:]],
            )

            # q projections overlap with the collective
            for i in range(NDT):
                proj_jt(i, qT[i][:])

        # AllGather return, v side: all 32 chunks. Kept off the scalar ring
        # so DMA issue doesn't steal ACT-queue slots from the exp stream.
        ago = ag_out.rearrange("(r x) -> r x", x=KV2)

        # head pair 0's kT reads go FIRST on the ring (attention needs them
        # immediately); the vf bulk follows.
        ktp = kv_scope.enter_context(tc.tile_pool(name="ktp", bufs=2))

        def load_kt(p_):
            kt = []
            for r in range(R):
                t_ = ktp.tile([P, TL], BF16, tag=f"kt{r}", name=f"kt{r}")
                agr_k = ago[r, 0:D * TL].rearrange("(a b) -> a b", b=TL)
                nc.sync.dma_start(t_[:], agr_k[P * p_:P * (p_ + 1), :])
                kt.append(t_)
            return kt

        kt0 = load_kt(0)
        for r in range(R):
            agr_v = ago[r, D * TL:].rearrange("(a b) -> a b", b=VW)
            for s in range(NTT):
                nc.sync.dma_start(vf[NTT * r + s][:],
                                  agr_v[P * s:P * (s + 1), :])

        # FFN weight prefetch: sequence AFTER the kv gather reads (a WAW
        # poison on one element) — the collective saturates HBM bandwidth,
        # and attention has ~200us of DMA-idle time to absorb these 14MB.
        nc.vector.tensor_copy(w1sb[0:1, 0, 0:1], vf[NCH - 1][0:1, 0:1])
        nc.gpsimd.dma_start(w1sb[:], w1h.ap())
        nc.vector.tensor_copy(w2sb[0:1, 0, 0:1], vf[NCH - 1][0:1, 0:1])
        nc.gpsimd.dma_start(w2sb[:], w2b.ap())

        # ------------------------------------------------------------------
        # Phase 2: attention, head pairs (row-packed score matmuls)
        # ------------------------------------------------------------------
        with ExitStack() as ph2:
            scp = ph2.enter_context(tc.tile_pool(name="scp", bufs=2, space="PSUM"))
            accp = ph2.enter_context(tc.tile_pool(name="accp", bufs=3, space="PSUM"))
            bcp = ph2.enter_context(tc.tile_pool(name="bcp", bufs=1, space="PSUM"))
            weip = ph2.enter_context(tc.tile_pool(name="weip", bufs=6))
            tailp = ph2.enter_context(tc.tile_pool(name="tailp", bufs=2))

            def finish_pair(pend):
                """Normalize a finished pair: reciprocal (slow, DVE) ->
                PE broadcast -> multiply into attnT. Emitted mid-way through
                the NEXT pair so the bc matmuls never stall the PE queue."""
                pp, num, dens = pend
                recb = tailp.tile([P, TL], F32, tag="recb", name="recb")
                for sub in (0, 1):
                    rec = tailp.tile([1, TL], F32R, tag=f"rec{sub}", name="rec")
                    with nc.allow_low_precision(reason="f32r recip, as baseline"):
                        nc.vector.reciprocal(rec[:], dens[sub][:].bitcast(F32R))
                    bc = bcp.tile([HD, TL], F32, tag="bc", name="bc")
                    nc.tensor.matmul(bc[:], ones_row[:, 0:HD], rec[:],
                                     start=True, stop=True)
                    nc.vector.tensor_copy(recb[sub * HD:(sub + 1) * HD, :], bc[:])
                nc.vector.tensor_tensor(attnT[pp][:], num[:].bitcast(F32R),
                                        recb[:].bitcast(F32R),
                                        mybir.AluOpType.mult)

            pend = None
            for p_ in range(H // 2):
                h0, h1 = 2 * p_, 2 * p_ + 1
                kt = kt0 if p_ == 0 else load_kt(p_)
                acc0 = accp.tile([HD + 1, TL], F32, tag="acc", name="acc0")
                acc1 = accp.tile([HD + 1, TL], F32, tag="acc", name="acc1")
                for c in range(NCH):
                    r, s = c // NTT, c % NTT
                    kT_c = kt[r][:, P * s:P * (s + 1)]
                    sc = scp.tile([P, 2 * TL], F32, tag="sc", name="sc")
                    nc.tensor.matmul(sc[:, 0:TL], kT_c[0:HD, :],
                                     qT[p_][0:HD, :], start=True, stop=True)
                    nc.tensor.matmul(sc[:, TL:2 * TL], kT_c[HD:P, :],
                                     qT[p_][HD:P, :], start=True, stop=True)
                    wei = weip.tile([P, 2 * TL], BF16, tag="wei", name="wei")
                    nc.scalar.activation(wei[:], sc[:], AFT.Exp, scale=SCALE)
                    nc.tensor.matmul(acc0[:],
                                     vf[c][:, (HD + 1) * h0:(HD + 1) * (h0 + 1)],
                                     wei[:, 0:TL],
                                     start=(c == 0), stop=(c == NCH - 1))
                    nc.tensor.matmul(acc1[:],
                                     vf[c][:, (HD + 1) * h1:(HD + 1) * (h1 + 1)],
                                     wei[:, TL:2 * TL],
                                     start=(c == 0), stop=(c == NCH - 1))
                    if c == 16 and pend is not None:
                        finish_pair(pend)
                        pend = None
                # Evacuate numerators + denominators to SBUF right away so
                # the acc PSUM banks free fast; the slow reciprocal and the
                # broadcast run later, off the critical path.
                num = tailp.tile([P, TL], F32, tag="num", name="num")
                nc.vector.tensor_copy(num[0:HD, :], acc0[0:HD, :])
                nc.vector.tensor_copy(num[HD:P, :], acc1[0:HD, :])
                dens = []
                for sub, acc in ((0, acc0), (1, acc1)):
                    den = tailp.tile([1, TL], F32, tag=f"den{sub}", name="den")
                    nc.vector.tensor_copy(den[:], acc[HD:HD + 1, :])
                    dens.append(den)
                pend = (p_, num, dens)
            finish_pair(pend)

        kv_scope.close()

        # ------------------------------------------------------------------
        # Phase 3: FFN1 (gelu) pipelined with FFN2 sweep A, then sweep B
        # ------------------------------------------------------------------
        hTp = top.enter_context(tc.tile_pool(name="hTp", bufs=1))
        hT = [hTp.tile([P, TL], BF16, name=f"hT{f}") for f in range(NFT)]

        with ExitStack() as ph3:
            ps1 = ph3.enter_context(tc.tile_pool(name="ps1", bufs=2, space="PSUM"))
            ps2 = ph3.enter_context(tc.tile_pool(name="ps2", bufs=1, space="PSUM"))
            outp = ph3.enter_context(tc.tile_pool(name="outp", bufs=1))
            out_sb = [outp.tile([P, D], F32, name=f"out{tt}") for tt in range(NTT)]

            acc2 = {}
            for tt in (0, 1):
                for o2 in range(2):
                    acc2[(tt, o2)] = ps2.tile([P, 384], F32, tag=f"g{tt}{o2}",
                                              name=f"acc2_{tt}_{o2}")
            for ft in range(NFT):
                ps = ps1.tile([P, TL], F32, tag="ps1t", name="ps1t")
                for d_ in range(NDT):
                    nc.tensor.matmul(ps[:], w1sb[:, d_, P * ft:P * (ft + 1)],
                                     attnT[d_][:],
                                     start=(d_ == 0), stop=(d_ == NDT - 1))
                nc.scalar.activation(hT[ft][:], ps[:], AFT.Gelu,
                                     bias=b1_sb[:, ft:ft + 1])
                # FFN2 sweep A accumulates as soon as each hT tile is ready
                for tt in (0, 1):
                    for o2 in range(2):
                        nc.tensor.matmul(acc2[(tt, o2)][:],
                                         hT[ft][:, P * tt:P * (tt + 1)],
                                         w2sb[:, ft, 384 * o2:384 * (o2 + 1)],
                                         start=(ft == 0), stop=False)
            for tt in (0, 1):
                for o2 in range(2):
                    sl = slice(384 * o2, 384 * (o2 + 1))
                    nc.tensor.matmul(acc2[(tt, o2)][:], ones_row[:], b2_sb[:, sl],
                                     start=False, stop=True)
                    nc.vector.tensor_copy(out_sb[tt][:, sl], acc2[(tt, o2)][:])
                nc.sync.dma_start(y.ap()[P * tt:P * (tt + 1), :], out_sb[tt][:])

            # sweep B (reuses the same 4 PSUM banks after sweep A evacuates)
            accB = {}
            for tt in (2, 3):
                for o2 in range(2):
                    accB[(tt, o2)] = ps2.tile([P, 384], F32, tag=f"g{tt - 2}{o2}",
                                              name=f"acc2_{tt}_{o2}")
            for ft in range(NFT):
                for tt in (2, 3):
                    for o2 in range(2):
                        nc.tensor.matmul(accB[(tt, o2)][:],
                                         hT[ft][:, P * tt:P * (tt + 1)],
                                         w2sb[:, ft, 384 * o2:384 * (o2 + 1)],
                                         start=(ft == 0), stop=False)
            for tt in (2, 3):
                for o2 in range(2):
                    sl = slice(384 * o2, 384 * (o2 + 1))
                    nc.tensor.matmul(accB[(tt, o2)][:], ones_row[:], b2_sb[:, sl],
                                     start=False, stop=True)
                    nc.vector.tensor_copy(out_sb[tt][:, sl], accB[(tt, o2)][:])
                nc.sync.dma_start(y.ap()[P * tt:P * (tt + 1), :], out_sb[tt][:])

    return nc


def _get_nc():
    if "nc" not in _NC_CACHE:
        _NC_CACHE["nc"] = _build_nc()
    return _NC_CACHE["nc"]


def _prep_common(inputs):
    w_qkv = np.ascontiguousarray(np.asarray(inputs["w_qkv"], np.float32))
    w_ff1 = np.ascontiguousarray(np.asarray(inputs["w_ff1"], np.float32))
    common = {
        # q/k columns, bf16, d-tile-major: [128, 6, 1536]
        "wqk": np.ascontiguousarray(
            w_qkv[:, :2 * D].reshape(NDT, P, 2 * D).transpose(1, 0, 2)
        ).astype(ml_dtypes.bfloat16),
        # v columns, bf16, d-tile-major: [128, 6, 768]
        "wv": np.ascontiguousarray(
            w_qkv[:, 2 * D:].reshape(NDT, P, D).transpose(1, 0, 2)
        ).astype(ml_dtypes.bfloat16),
        # w_ff1 d-tile-major, bf16: [128, 6, 3072]
        "w1h": np.ascontiguousarray(
            w_ff1.reshape(NDT, P, DFF).transpose(1, 0, 2)
        ).astype(ml_dtypes.bfloat16),
        # w_ff2 ff-tile-major, bf16: [128, 24, 768]
        "w2b": np.ascontiguousarray(
            np.asarray(inputs["w_ff2"], np.float32)
            .reshape(NFT, P, D).transpose(1, 0, 2)).astype(ml_dtypes.bfloat16),
        "b_qkv": np.ascontiguousarray(np.asarray(inputs["b_qkv"], np.float32)),
        "b_ff1": np.ascontiguousarray(np.asarray(inputs["b_ff1"], np.float32)),
        "b_ff2": np.ascontiguousarray(np.asarray(inputs["b_ff2"], np.float32)),
    }
    return common


def run_sharded(inputs, **run_kwargs):
    """Run the SPMD kernel; returns (full_output [1,4096,768], BassKernelResults)."""
    x = np.ascontiguousarray(np.asarray(inputs["x"], dtype=np.float32))
    assert x.shape == (1, T, D), x.shape
    common = _prep_common(inputs)
    in_maps = []
    for r in range(R):
        m = dict(common)
        xr = x[0, TL * r:TL * (r + 1), :]  # [512, 768]
        xT = np.ascontiguousarray(xr.T.reshape(NDT, P, TL).transpose(1, 0, 2))
        m["xTb"] = xT.astype(ml_dtypes.bfloat16)
        in_maps.append(m)
    nc = _get_nc()
    res = run_bass_kernel_spmd(nc, in_maps, core_ids=list(range(R)), **run_kwargs)
    out = np.concatenate([res.results[r]["y"] for r in range(R)], axis=0)
    return out.reshape(1, T, D), res


def kernel(**inputs):
    out, _ = run_sharded(inputs)
    return out



# revision 13
# speedup vs baseline: 1.0417x; 1.0417x over previous
"""Trainium2 Bass kernel for a single-layer MHA + FFN transformer block.

Reference computation (for x: [1, 4096, 768], 12 heads, dff=3072):
    qkv = (x @ w_qkv + b_qkv)  -> q, k, v       [t, 768] each
    scores = q k^T / sqrt(768) ; wei = softmax(scores)
    attn = wei @ v  (concat heads)              [t, 768]
    h = gelu(attn @ w_ff1 + b_ff1)              [t, 3072]
    out = h @ w_ff2 + b_ff2                     [t, 768]

Sharding: sequence-parallel over the 4096 tokens across 8 NeuronCores
(512 rows each). Every core computes q/k/v for its own rows, the k/v
blocks are exchanged with one bf16 AllGather, then each core runs full
attention for its 512 query rows over all 4096 keys plus the whole FFN
for its rows. Outputs are concatenated on the host.

Host-side prep (all cheap numpy): x is pre-transposed per core (bf16 for
the q/k path, f32 for the v path), w_qkv's q/k columns are pre-cast to
bf16 and laid out d-tile-major so every weight load is a contiguous DMA,
and w_ff1 is laid out so the whole matrix loads with one DMA.

The v tiles carry a per-head ones column through the AllGather (width
65*12=780) so softmax denominators ride along in the wei@v matmuls and
the gathered v reads back with fully contiguous DMAs. All 8 ranks' k/v
(including our own) are read back from the gather output so the program
stays rank-uniform.

Precision: q/k projections run bf16 (scores are bf16 anyway), v and the
FFN run fp32r (full PE rate at free-dim>=256); softmax skips
max-subtraction because the logits here are bounded by ~0.6.
"""

import json as _json
import math

import numpy as np
import ml_dtypes

import concourse.bass as bass
import concourse.mybir as mybir
import concourse.tile as tile
from concourse.bass_utils import run_bass_kernel_spmd

# ---------------------------------------------------------------------------
# Workaround: the pinned walrus build only supports ONE embedded semaphore
# wait per instruction, but Tile's sem assigner attaches several. Split the
# excess onto standalone EventSemaphore instructions (pure waits) inserted
# just before the over-subscribed instruction (same engine => same program
# order, identical semantics).
# ---------------------------------------------------------------------------
_MAX_WAITS = 1
_ctr = [0]
if not getattr(bass.Bass, "_multiwait_patched", False):
    _orig_to_json_bytes = bass.Bass.to_json_bytes

    def _split_multiwait_json_bytes(self):
        bir = _json.loads(_orig_to_json_bytes(self))
        for f in bir["functions"]:
            for b in f["blocks"]:
                new_insts = []
                for inst in b["instructions"]:
                    si = inst.get("sync_info")
                    waits = si.get("on_wait", []) if si else []
                    if len(waits) > _MAX_WAITS:
                        excess, keep = waits[:-_MAX_WAITS], waits[-_MAX_WAITS:]
                        for k in range(0, len(excess), _MAX_WAITS):
                            _ctr[0] += 1
                            new_insts.append({
                                "debug": inst.get("debug", 0),
                                "engine": inst["engine"],
                                "ins": [], "outs": [],
                                "name": "I-waitsplit-%d" % _ctr[0],
                                "opcode": "EventSemaphore",
                                "sync_info": {"on_update": [],
                                              "on_wait": excess[k:k + _MAX_WAITS]},
                            })
                        si["on_wait"] = keep
                    new_insts.append(inst)
                b["instructions"] = new_insts
        return _json.dumps(bir).encode()

    bass.Bass.to_json_bytes = _split_multiwait_json_bytes
    bass.Bass._multiwait_patched = True

F32 = mybir.dt.float32
F32R = mybir.dt.float32r
BF16 = mybir.dt.bfloat16
AFT = mybir.ActivationFunctionType

R = 8          # cores
T = 4096       # sequence length
TL = T // R    # rows per core (512)
D = 768
H = 12
HD = D // H    # 64
DFF = 4 * D    # 3072
P = 128
NDT = D // P   # 6 d-tiles
NTT = TL // P  # 4 local t-tiles
NFT = DFF // P  # 24 dff tiles
NCH = T // P   # 32 global key chunks
SCALE = 1.0 / math.sqrt(D)
VW = H * (HD + 1)      # 780: v tile width with a ones column per head
KV2 = D * TL + TL * VW  # 792576 elems in the AllGather payload per rank

_NC_CACHE = {}


def _build_nc():
    nc = bass.Bass(num_devices=R)
    # host-prepped per-core inputs
    xTb = nc.declare_dram_parameter("xTb", [P, NDT, TL], BF16, isOutput=False)
    # host-prepped common weights (k and q split so the k-path DMA can be
    # prioritized: k/v projections gate the AllGather trigger)
    wkc = nc.declare_dram_parameter("wkc", [P, NDT, D], BF16, isOutput=False)
    wqc = nc.declare_dram_parameter("wqc", [P, NDT, D], BF16, isOutput=False)
    wv = nc.declare_dram_parameter("wv", [P, NDT, D], BF16, isOutput=False)
    w1h = nc.declare_dram_parameter("w1h", [P, NDT, DFF], BF16, isOutput=False)
    w2b = nc.declare_dram_parameter("w2b", [P, NFT, D], BF16, isOutput=False)
    b_qkv = nc.declare_dram_parameter("b_qkv", [3 * D], F32, isOutput=False)
    b_ff1 = nc.declare_dram_parameter("b_ff1", [DFF], F32, isOutput=False)
    b_ff2 = nc.declare_dram_parameter("b_ff2", [D], F32, isOutput=False)
    # per-core indices of the 7 REMOTE ranks' rows in the gathered k/v
    # buffers (processing order rank+1, rank+2, ... mod 8). cols 0..41: k
    # row ids per (pair, remote) [128 rows each]; cols 42..69: v row ids
    # per remote chunk.
    kvidx = nc.declare_dram_parameter("kvidx", [P, 7 * NDT + 28], mybir.dt.int32,
                                      isOutput=False)
    y = nc.declare_dram_parameter("y", [TL, D], F32, isOutput=True)

    from contextlib import ExitStack

    with tile.TileContext(nc) as tc, ExitStack() as top:
        const = top.enter_context(tc.tile_pool(name="const", bufs=1))
        dramp = top.enter_context(tc.tile_pool(name="dramp", bufs=1, space="DRAM"))
        persist = top.enter_context(tc.tile_pool(name="persist", bufs=1))

        ones_dram = nc.inline_tensor(np.ones((1, P), np.float32), name="ones_const")
        ones_row = const.tile([1, P], F32R, name="ones_row")
        nc.sync.dma_start(ones_row[:], ones_dram.ap().bitcast(F32R))
        # head-pair selection for the denominator broadcast: row 0 -> cols
        # 0..63 (even head), row 32 -> cols 64..127 (odd head). Rows 0/32
        # because engine partition bases must be 32-aligned; the unused rows
        # are zero so garbage rec values there cannot leak through the matmul.
        sel_np = np.zeros((33, P), np.float32)
        sel_np[0, 0:HD] = 1.0
        sel_np[32, HD:P] = 1.0
        sel_dram = nc.inline_tensor(sel_np, name="sel33_const")
        sel33 = const.tile([33, P], F32R, name="sel33")
        nc.sync.dma_start(sel33[:], sel_dram.ap().bitcast(F32R))

        bq_sb = const.tile([P, 2 * NDT], F32, name="bq_sb")
        nc.gpsimd.dma_start(
            bq_sb[:], b_qkv.ap()[0:2 * D].rearrange("(o p) -> p o", p=P))
        bv_sb = const.tile([1, D], F32R, name="bv_sb")
        nc.gpsimd.dma_start(bv_sb[:], b_qkv.ap()[None, 2 * D:3 * D].bitcast(F32R))
        b1_sb = const.tile([P, NFT], F32, name="b1_sb")
        nc.gpsimd.dma_start(b1_sb[:], b_ff1.ap().rearrange("(o p) -> p o", p=P))
        b2_sb = const.tile([1, D], F32R, name="b2_sb")
        nc.gpsimd.dma_start(b2_sb[:], b_ff2.ap()[None, :].bitcast(F32R))

        # identity for seeding phase-B accumulators from phase-A partials
        i65_dram = nc.inline_tensor(
            np.eye(HD + 1, dtype=np.float32), name="i65_const")
        i65 = const.tile([HD + 1, HD + 1], BF16, name="i65")
        nc.gpsimd.dma_start(i65[:], i65_dram.ap())

        kvidx_sb = const.tile([P, 7 * NDT + 28], mybir.dt.int32, name="kvidx_sb")
        nc.gpsimd.dma_start(kvidx_sb[:], kvidx.ap())

        # preload the exp activation table while phase 1 is DMA-bound
        warmup = const.tile([P, 1], F32, name="warmup")
        nc.scalar.activation(warmup[:], bq_sb[:, 0:1], AFT.Exp)

        # split k / v AllGathers: k completes first so phase-B scores start
        # while the v gather is still on the wire
        agk_in = dramp.tile([D * TL], BF16, name="agk_in")
        agk_out = dramp.tile([R * D * TL], BF16, addr_space="Shared",
                             name="agk_out")
        agv_in = dramp.tile([TL * VW], BF16, name="agv_in")
        agv_out = dramp.tile([R * TL * VW], BF16, addr_space="Shared",
                             name="agv_out")

        attnT = [persist.tile([P, TL], BF16, name=f"attnT{i}") for i in range(NDT)]
        w1sb = persist.tile([P, NDT, DFF], BF16, name="w1sb")
        w2sb = persist.tile([P, NFT, D], BF16, name="w2sb")

        NRC = NCH - NTT  # 28 remote chunks

        kv_scope = top.enter_context(ExitStack())
        kvp = kv_scope.enter_context(tc.tile_pool(name="kvp", bufs=1))
        qT = [kvp.tile([P, TL], BF16, name=f"qT{i}") for i in range(NDT)]
        kT_loc = [kvp.tile([P, TL], BF16, name=f"kTl{i}") for i in range(NDT)]
        v_loc = [kvp.tile([P, VW], BF16, name=f"vl{t}") for t in range(NTT)]
        vf = [kvp.tile([P, VW], BF16, name=f"vf{c}") for c in range(NRC)]
        # phase-A partial numerators+denominators (row 64), one per head
        locn = [[kvp.tile([HD + 1, TL], BF16, name=f"loc{p}_{s}")
                 for s in range(2)] for p in range(H // 2)]

        # ------------------------------------------------------------------
        # Phase 1: QKV projections straight from host-transposed x
        # ------------------------------------------------------------------
        with ExitStack() as ph1:
            xp = ph1.enter_context(tc.tile_pool(name="xp", bufs=1))
            psQ = ph1.enter_context(tc.tile_pool(name="psQ", bufs=2, space="PSUM"))
            psV = ph1.enter_context(tc.tile_pool(name="psV", bufs=2, space="PSUM"))

            xtb_sb = xp.tile([P, NDT, TL], BF16, name="xtb")
            nc.sync.dma_start(xtb_sb[:], xTb.ap())
            wk_sb = xp.tile([P, NDT, D], BF16, name="wk_sb")
            nc.sync.dma_start(wk_sb[:], wkc.ap())
            wv_sb = xp.tile([P, NDT, D], BF16, name="wv_sb")
            nc.scalar.dma_start(wv_sb[:], wv.ap())
            wq_sb = xp.tile([P, NDT, D], BF16, name="wq_sb")

            def proj_jt(w_sb, jt, bcol, out_ap):
                """qkv^T tile for channel block jt (0..5) of weight w_sb."""
                ps = psQ.tile([P, TL], F32, tag="psq", name="psq")
                for d_ in range(NDT):
                    nc.tensor.matmul(ps[:], w_sb[:, d_, P * jt:P * (jt + 1)],
                                     xtb_sb[:, d_, :],
                                     start=(d_ == 0), stop=(d_ == NDT - 1))
                nc.vector.tensor_scalar_add(out_ap, ps[:], bq_sb[:, bcol:bcol + 1])

            # k first (AllGather input): bias cols 6..11 of bq_sb.
            # Stage the local k/v blocks in vf tiles — the gather readback
            # overwrites them later (Tile serializes write-after-read).
            ag_k = ag_in[0:D * TL].rearrange("(a b) -> a b", b=TL)
            kT_loc = [vf[i][:, 0:TL] for i in range(NDT)]
            for i in range(NDT):
                proj_jt(wk_sb, i, NDT + i, kT_loc[i])
                nc.sync.dma_start(ag_k[P * i:P * (i + 1), :], kT_loc[i])

            # v in [t, j] orientation with the per-head ones columns baked in
            ag_v = ag_in[D * TL:].rearrange("(a b) -> a b", b=VW)
            v_loc = [vf[NDT + t] for t in range(NTT)]
            for tt in range(NTT):
                vfv = v_loc[tt].rearrange("p (h e) -> p h e", e=HD + 1)
                nc.vector.memset(vfv[:, :, HD:HD + 1], 1.0)
                for o2 in range(2):
                    sl = slice(384 * o2, 384 * (o2 + 1))
                    ps = psV.tile([P, 384], F32, tag="psv", name="psv")
                    for d_ in range(NDT):
                        nc.tensor.matmul(ps[:],
                                         xtb_sb[:, d_, P * tt:P * (tt + 1)],
                                         wv_sb[:, d_, sl],
                                         start=(d_ == 0), stop=False)
                    nc.tensor.matmul(ps[:], ones_row[:], bv_sb[:, sl],
                                     start=False, stop=True)
                    dst = vfv[:, 6 * o2:6 * (o2 + 1), 0:HD]
                    nc.vector.tensor_copy(
                        dst, ps[:].rearrange("p (h e) -> p h e", e=HD))
                nc.sync.dma_start(ag_v[P * tt:P * (tt + 1), :], v_loc[tt][:])

            nc.gpsimd.collective_compute(
                "AllGather", mybir.AluOpType.bypass,
                replica_groups=[list(range(R))],
                ins=[ag_in[:]], outs=[ag_out[:]],
            )

            # q projections overlap with the collective (wq DMA deferred so
            # it never competes with the k/v path that gates the AllGather)
            nc.scalar.dma_start(wq_sb[:], wqc.ap())
            for i in range(NDT):
                proj_jt(wq_sb, i, i, qT[i][:])

        # AllGather return, v side: all 32 chunks. Kept off the scalar ring
        # so DMA issue doesn't steal ACT-queue slots from the exp stream.
        ago = ag_out.rearrange("(r x) -> r x", x=KV2)

        # head pair 0's kT reads go FIRST on the ring (attention needs them
        # immediately); the vf bulk follows.
        ktp = kv_scope.enter_context(tc.tile_pool(name="ktp", bufs=2))

        def load_kt(p_):
            kt = []
            for r in range(R):
                t_ = ktp.tile([P, TL], BF16, tag=f"kt{r}", name=f"kt{r}")
                agr_k = ago[r, 0:D * TL].rearrange("(a b) -> a b", b=TL)
                nc.sync.dma_start(t_[:], agr_k[P * p_:P * (p_ + 1), :])
                kt.append(t_)
            return kt

        kt0 = load_kt(0)
        for r in range(R):
            agr_v = ago[r, D * TL:].rearrange("(a b) -> a b", b=VW)
            for s in range(NTT):
                nc.sync.dma_start(vf[NTT * r + s][:],
                                  agr_v[P * s:P * (s + 1), :])

        # FFN weight prefetch: sequence AFTER the kv gather reads (a WAW
        # poison on one element) — the collective saturates HBM bandwidth,
        # and attention has ~200us of DMA-idle time to absorb these 14MB.
        nc.vector.tensor_copy(w1sb[0:1, 0, 0:1], vf[NCH - 1][0:1, 0:1])
        nc.gpsimd.dma_start(w1sb[:], w1h.ap())
        nc.vector.tensor_copy(w2sb[0:1, 0, 0:1], vf[NCH - 1][0:1, 0:1])
        nc.gpsimd.dma_start(w2sb[:], w2b.ap())

        # ------------------------------------------------------------------
        # Phase 2: attention, head pairs (row-packed score matmuls)
        # ------------------------------------------------------------------
        with ExitStack() as ph2:
            scp = ph2.enter_context(tc.tile_pool(name="scp", bufs=2, space="PSUM"))
            accp = ph2.enter_context(tc.tile_pool(name="accp", bufs=3, space="PSUM"))
            bcp = ph2.enter_context(tc.tile_pool(name="bcp", bufs=1, space="PSUM"))
            weip = ph2.enter_context(tc.tile_pool(name="weip", bufs=6))
            tailp = ph2.enter_context(tc.tile_pool(name="tailp", bufs=2))

            def finish_pair(pend):
                """Normalize a finished pair: ONE merged reciprocal [2,TL]
                (halves the slow DVE recip cost) -> ONE sel2 broadcast matmul
                (K=2 picks rec row 0 for head-0 partitions, row 1 for head-1)
                -> multiply into attnT straight from PSUM. Emitted early in
                the NEXT pair so nothing lands on a pair boundary."""
                pp, num, den2 = pend
                rec2 = tailp.tile([33, TL], F32R, tag="rec2", name="rec2")
                with nc.allow_low_precision(reason="f32r recip, as baseline"):
                    nc.vector.reciprocal(rec2[:], den2[:].bitcast(F32R))
                bc = bcp.tile([P, TL], F32, tag="bc", name="bc")
                nc.tensor.matmul(bc[:], sel33[:, 0:P], rec2[:],
                                 start=True, stop=True)
                nc.vector.tensor_tensor(attnT[pp][:], num[:].bitcast(F32R),
                                        bc[:].bitcast(F32R),
                                        mybir.AluOpType.mult)

            pend = None
            for p_ in range(H // 2):
                h0, h1 = 2 * p_, 2 * p_ + 1
                kt = kt0 if p_ == 0 else load_kt(p_)
                acc0 = accp.tile([HD + 1, TL], F32, tag="acc", name="acc0")
                acc1 = accp.tile([HD + 1, TL], F32, tag="acc", name="acc1")
                for c in range(NCH):
                    r, s = c // NTT, c % NTT
                    kT_c = kt[r][:, P * s:P * (s + 1)]
                    sc = scp.tile([P, 2 * TL], F32, tag="sc", name="sc")
                    nc.tensor.matmul(sc[:, 0:TL], kT_c[0:HD, :],
                                     qT[p_][0:HD, :], start=True, stop=True)
                    nc.tensor.matmul(sc[:, TL:2 * TL], kT_c[HD:P, :],
                                     qT[p_][HD:P, :], start=True, stop=True)
                    wei = weip.tile([P, 2 * TL], BF16, tag="wei", name="wei")
                    nc.scalar.activation(wei[:], sc[:], AFT.Exp, scale=SCALE)
                    nc.tensor.matmul(acc0[:],
                                     vf[c][:, (HD + 1) * h0:(HD + 1) * (h0 + 1)],
                                     wei[:, 0:TL],
                                     start=(c == 0), stop=(c == NCH - 1))
                    nc.tensor.matmul(acc1[:],
                                     vf[c][:, (HD + 1) * h1:(HD + 1) * (h1 + 1)],
                                     wei[:, TL:2 * TL],
                                     start=(c == 0), stop=(c == NCH - 1))
                    if c == 8 and pend is not None:
                        finish_pair(pend)
                        pend = None
                # Evacuate numerators + denominators to SBUF right away so
                # the acc PSUM banks free fast; the slow reciprocal and the
                # broadcast run later, off the critical path.
                num = tailp.tile([P, TL], F32, tag="num", name="num")
                nc.vector.tensor_copy(num[0:HD, :], acc0[0:HD, :])
                nc.vector.tensor_copy(num[HD:P, :], acc1[0:HD, :])
                den2 = tailp.tile([33, TL], F32, tag="den2", name="den2")
                nc.vector.memset(den2[:], 1.0)
                nc.vector.tensor_copy(den2[0:1, :], acc0[HD:HD + 1, :])
                nc.vector.tensor_copy(den2[32:33, :], acc1[HD:HD + 1, :])
                pend = (p_, num, den2)
            finish_pair(pend)

        kv_scope.close()

        # ------------------------------------------------------------------
        # Phase 3: FFN1 (gelu) pipelined with FFN2 sweep A, then sweep B
        # ------------------------------------------------------------------
        hTp = top.enter_context(tc.tile_pool(name="hTp", bufs=1))
        hT = [hTp.tile([P, TL], BF16, name=f"hT{f}") for f in range(NFT)]

        with ExitStack() as ph3:
            ps1 = ph3.enter_context(tc.tile_pool(name="ps1", bufs=2, space="PSUM"))
            ps2 = ph3.enter_context(tc.tile_pool(name="ps2", bufs=1, space="PSUM"))
            outp = ph3.enter_context(tc.tile_pool(name="outp", bufs=1))
            out_sb = [outp.tile([P, D], F32, name=f"out{tt}") for tt in range(NTT)]

            acc2 = {}
            for tt in (0, 1):
                for o2 in range(2):
                    acc2[(tt, o2)] = ps2.tile([P, 384], F32, tag=f"g{tt}{o2}",
                                              name=f"acc2_{tt}_{o2}")
            for ft in range(NFT):
                ps = ps1.tile([P, TL], F32, tag="ps1t", name="ps1t")
                for d_ in range(NDT):
                    nc.tensor.matmul(ps[:], w1sb[:, d_, P * ft:P * (ft + 1)],
                                     attnT[d_][:],
                                     start=(d_ == 0), stop=(d_ == NDT - 1))
                nc.scalar.activation(hT[ft][:], ps[:], AFT.Gelu,
                                     bias=b1_sb[:, ft:ft + 1])
                # FFN2 sweep A accumulates as soon as each hT tile is ready
                for tt in (0, 1):
                    for o2 in range(2):
                        nc.tensor.matmul(acc2[(tt, o2)][:],
                                         hT[ft][:, P * tt:P * (tt + 1)],
                                         w2sb[:, ft, 384 * o2:384 * (o2 + 1)],
                                         start=(ft == 0), stop=False)
            for tt in (0, 1):
                for o2 in range(2):
                    sl = slice(384 * o2, 384 * (o2 + 1))
                    nc.tensor.matmul(acc2[(tt, o2)][:], ones_row[:], b2_sb[:, sl],
                                     start=False, stop=True)
                    nc.vector.tensor_copy(out_sb[tt][:, sl], acc2[(tt, o2)][:])
                nc.sync.dma_start(y.ap()[P * tt:P * (tt + 1), :], out_sb[tt][:])

            # sweep B (reuses the same 4 PSUM banks after sweep A evacuates)
            accB = {}
            for tt in (2, 3):
                for o2 in range(2):
                    accB[(tt, o2)] = ps2.tile([P, 384], F32, tag=f"g{tt - 2}{o2}",
                                              name=f"acc2_{tt}_{o2}")
            for ft in range(NFT):
                for tt in (2, 3):
                    for o2 in range(2):
                        nc.tensor.matmul(accB[(tt, o2)][:],
                                         hT[ft][:, P * tt:P * (tt + 1)],
                                         w2sb[:, ft, 384 * o2:384 * (o2 + 1)],
                                         start=(ft == 0), stop=False)
            for tt in (2, 3):
                for o2 in range(2):
                    sl = slice(384 * o2, 384 * (o2 + 1))
                    nc.tensor.matmul(accB[(tt, o2)][:], ones_row[:], b2_sb[:, sl],
                                     start=False, stop=True)
                    nc.vector.tensor_copy(out_sb[tt][:, sl], accB[(tt, o2)][:])
                nc.sync.dma_start(y.ap()[P * tt:P * (tt + 1), :], out_sb[tt][:])

    return nc


def _get_nc():
    if "nc" not in _NC_CACHE:
        _NC_CACHE["nc"] = _build_nc()
    return _NC_CACHE["nc"]


def _prep_common(inputs):
    w_qkv = np.ascontiguousarray(np.asarray(inputs["w_qkv"], np.float32))
    w_ff1 = np.ascontiguousarray(np.asarray(inputs["w_ff1"], np.float32))
    common = {
        # q columns, bf16, d-tile-major: [128, 6, 768]
        "wqc": np.ascontiguousarray(
            w_qkv[:, 0:D].reshape(NDT, P, D).transpose(1, 0, 2)
        ).astype(ml_dtypes.bfloat16),
        # k columns, bf16, d-tile-major: [128, 6, 768]
        "wkc": np.ascontiguousarray(
            w_qkv[:, D:2 * D].reshape(NDT, P, D).transpose(1, 0, 2)
        ).astype(ml_dtypes.bfloat16),
        # v columns, bf16, d-tile-major: [128, 6, 768]
        "wv": np.ascontiguousarray(
            w_qkv[:, 2 * D:].reshape(NDT, P, D).transpose(1, 0, 2)
        ).astype(ml_dtypes.bfloat16),
        # w_ff1 d-tile-major, bf16: [128, 6, 3072]
        "w1h": np.ascontiguousarray(
            w_ff1.reshape(NDT, P, DFF).transpose(1, 0, 2)
        ).astype(ml_dtypes.bfloat16),
        # w_ff2 ff-tile-major, bf16: [128, 24, 768]
        "w2b": np.ascontiguousarray(
            np.asarray(inputs["w_ff2"], np.float32)
            .reshape(NFT, P, D).transpose(1, 0, 2)).astype(ml_dtypes.bfloat16),
        "b_qkv": np.ascontiguousarray(np.asarray(inputs["b_qkv"], np.float32)),
        "b_ff1": np.ascontiguousarray(np.asarray(inputs["b_ff1"], np.float32)),
        "b_ff2": np.ascontiguousarray(np.asarray(inputs["b_ff2"], np.float32)),
    }
    return common


def run_sharded(inputs, **run_kwargs):
    """Run the SPMD kernel; returns (full_output [1,4096,768], BassKernelResults)."""
    x = np.ascontiguousarray(np.asarray(inputs["x"], dtype=np.float32))
    assert x.shape == (1, T, D), x.shape
    common = _prep_common(inputs)
    in_maps = []
    for r in range(R):
        m = dict(common)
        xr = x[0, TL * r:TL * (r + 1), :]  # [512, 768]
        xT = np.ascontiguousarray(xr.T.reshape(NDT, P, TL).transpose(1, 0, 2))
        m["xTb"] = xT.astype(ml_dtypes.bfloat16)
        in_maps.append(m)
    nc = _get_nc()
    res = run_bass_kernel_spmd(nc, in_maps, core_ids=list(range(R)), **run_kwargs)
    out = np.concatenate([res.results[r]["y"] for r in range(R)], axis=0)
    return out.reshape(1, T, D), res


def kernel(**inputs):
    out, _ = run_sharded(inputs)
    return out



# revision 16
# speedup vs baseline: 1.0469x; 1.0050x over previous
"""Trainium2 Bass kernel for a single-layer MHA + FFN transformer block.

Reference computation (for x: [1, 4096, 768], 12 heads, dff=3072):
    qkv = (x @ w_qkv + b_qkv)  -> q, k, v       [t, 768] each
    scores = q k^T / sqrt(768) ; wei = softmax(scores)
    attn = wei @ v  (concat heads)              [t, 768]
    h = gelu(attn @ w_ff1 + b_ff1)              [t, 3072]
    out = h @ w_ff2 + b_ff2                     [t, 768]

Sharding: sequence-parallel over the 4096 tokens across 8 NeuronCores
(512 rows each). Every core computes q/k/v for its own rows, the k/v
blocks are exchanged with one bf16 AllGather, then each core runs full
attention for its 512 query rows over all 4096 keys plus the whole FFN
for its rows. Outputs are concatenated on the host.

Host-side prep (all cheap numpy): x is pre-transposed per core (bf16 for
the q/k path, f32 for the v path), w_qkv's q/k columns are pre-cast to
bf16 and laid out d-tile-major so every weight load is a contiguous DMA,
and w_ff1 is laid out so the whole matrix loads with one DMA.

The v tiles carry a per-head ones column through the AllGather (width
65*12=780) so softmax denominators ride along in the wei@v matmuls and
the gathered v reads back with fully contiguous DMAs. All 8 ranks' k/v
(including our own) are read back from the gather output so the program
stays rank-uniform.

Precision: q/k projections run bf16 (scores are bf16 anyway), v and the
FFN run fp32r (full PE rate at free-dim>=256); softmax skips
max-subtraction because the logits here are bounded by ~0.6.
"""

import json as _json
import math

import numpy as np
import ml_dtypes

import concourse.bass as bass
import concourse.mybir as mybir
import concourse.tile as tile
from concourse.bass_utils import run_bass_kernel_spmd

# ---------------------------------------------------------------------------
# Workaround: the pinned walrus build only supports ONE embedded semaphore
# wait per instruction, but Tile's sem assigner attaches several. Split the
# excess onto standalone EventSemaphore instructions (pure waits) inserted
# just before the over-subscribed instruction (same engine => same program
# order, identical semantics).
# ---------------------------------------------------------------------------
_MAX_WAITS = 1
_ctr = [0]
if not getattr(bass.Bass, "_multiwait_patched", False):
    _orig_to_json_bytes = bass.Bass.to_json_bytes

    def _split_multiwait_json_bytes(self):
        bir = _json.loads(_orig_to_json_bytes(self))
        for f in bir["functions"]:
            for b in f["blocks"]:
                new_insts = []
                for inst in b["instructions"]:
                    si = inst.get("sync_info")
                    waits = si.get("on_wait", []) if si else []
                    if len(waits) > _MAX_WAITS:
                        excess, keep = waits[:-_MAX_WAITS], waits[-_MAX_WAITS:]
                        for k in range(0, len(excess), _MAX_WAITS):
                            _ctr[0] += 1
                            new_insts.append({
                                "debug": inst.get("debug", 0),
                                "engine": inst["engine"],
                                "ins": [], "outs": [],
                                "name": "I-waitsplit-%d" % _ctr[0],
                                "opcode": "EventSemaphore",
                                "sync_info": {"on_update": [],
                                              "on_wait": excess[k:k + _MAX_WAITS]},
                            })
                        si["on_wait"] = keep
                    new_insts.append(inst)
                b["instructions"] = new_insts
        return _json.dumps(bir).encode()

    bass.Bass.to_json_bytes = _split_multiwait_json_bytes
    bass.Bass._multiwait_patched = True

F32 = mybir.dt.float32
F32R = mybir.dt.float32r
BF16 = mybir.dt.bfloat16
AFT = mybir.ActivationFunctionType

R = 8          # cores
T = 4096       # sequence length
TL = T // R    # rows per core (512)
D = 768
H = 12
HD = D // H    # 64
DFF = 4 * D    # 3072
P = 128
NDT = D // P   # 6 d-tiles
NTT = TL // P  # 4 local t-tiles
NFT = DFF // P  # 24 dff tiles
NCH = T // P   # 32 global key chunks
SCALE = 1.0 / math.sqrt(D)
VW = H * (HD + 1)      # 780: v tile width with a ones column per head
KV2 = D * TL + TL * VW  # 792576 elems in the AllGather payload per rank

_NC_CACHE = {}


def _build_nc():
    nc = bass.Bass(num_devices=R)
    # host-prepped per-core inputs
    xTb = nc.declare_dram_parameter("xTb", [P, NDT, TL], BF16, isOutput=False)
    # host-prepped common weights (k and q split so the k-path DMA can be
    # prioritized: k/v projections gate the AllGather trigger)
    wkc = nc.declare_dram_parameter("wkc", [P, NDT, D], BF16, isOutput=False)
    wqc = nc.declare_dram_parameter("wqc", [P, NDT, D], BF16, isOutput=False)
    wv = nc.declare_dram_parameter("wv", [P, NDT, D], BF16, isOutput=False)
    w1h = nc.declare_dram_parameter("w1h", [P, NDT, DFF], BF16, isOutput=False)
    w2b = nc.declare_dram_parameter("w2b", [P, NFT, D], BF16, isOutput=False)
    b_qkv = nc.declare_dram_parameter("b_qkv", [3 * D], F32, isOutput=False)
    b_ff1 = nc.declare_dram_parameter("b_ff1", [DFF], F32, isOutput=False)
    b_ff2 = nc.declare_dram_parameter("b_ff2", [D], F32, isOutput=False)
    # per-core indices of the 7 REMOTE ranks' rows in the gathered k/v
    # buffers (processing order rank+1, rank+2, ... mod 8). cols 0..41: k
    # row ids per (pair, remote) [128 rows each]; cols 42..69: v row ids
    # per remote chunk.
    kvidx = nc.declare_dram_parameter("kvidx", [P, 7 * NDT + 28], mybir.dt.int32,
                                      isOutput=False)
    y = nc.declare_dram_parameter("y", [TL, D], F32, isOutput=True)

    from contextlib import ExitStack

    with tile.TileContext(nc) as tc, ExitStack() as top:
        const = top.enter_context(tc.tile_pool(name="const", bufs=1))
        dramp = top.enter_context(tc.tile_pool(name="dramp", bufs=1, space="DRAM"))
        persist = top.enter_context(tc.tile_pool(name="persist", bufs=1))

        ones_dram = nc.inline_tensor(np.ones((1, P), np.float32), name="ones_const")
        ones_row = const.tile([1, P], F32R, name="ones_row")
        nc.sync.dma_start(ones_row[:], ones_dram.ap().bitcast(F32R))
        # head-pair selection for the denominator broadcast: row 0 -> cols
        # 0..63 (even head), row 32 -> cols 64..127 (odd head). Rows 0/32
        # because engine partition bases must be 32-aligned; the unused rows
        # are zero so garbage rec values there cannot leak through the matmul.
        sel_np = np.zeros((33, P), np.float32)
        sel_np[0, 0:HD] = 1.0
        sel_np[32, HD:P] = 1.0
        sel_dram = nc.inline_tensor(sel_np, name="sel33_const")
        sel33 = const.tile([33, P], F32R, name="sel33")
        nc.sync.dma_start(sel33[:], sel_dram.ap().bitcast(F32R))

        bq_sb = const.tile([P, 2 * NDT], F32, name="bq_sb")
        nc.gpsimd.dma_start(
            bq_sb[:], b_qkv.ap()[0:2 * D].rearrange("(o p) -> p o", p=P))
        bv_sb = const.tile([1, D], F32R, name="bv_sb")
        nc.gpsimd.dma_start(bv_sb[:], b_qkv.ap()[None, 2 * D:3 * D].bitcast(F32R))
        b1_sb = const.tile([P, NFT], F32, name="b1_sb")
        nc.gpsimd.dma_start(b1_sb[:], b_ff1.ap().rearrange("(o p) -> p o", p=P))
        b2_sb = const.tile([1, D], F32R, name="b2_sb")
        nc.gpsimd.dma_start(b2_sb[:], b_ff2.ap()[None, :].bitcast(F32R))

        # identity for seeding phase-B accumulators from phase-A partials
        i65_dram = nc.inline_tensor(
            np.eye(HD + 1, dtype=np.float32), name="i65_const")
        i65 = const.tile([HD + 1, HD + 1], BF16, name="i65")
        nc.gpsimd.dma_start(i65[:], i65_dram.ap())

        kvidx_sb = const.tile([P, 7 * NDT + 28], mybir.dt.int32, name="kvidx_sb")
        nc.gpsimd.dma_start(kvidx_sb[:], kvidx.ap())

        # preload the exp activation table while phase 1 is DMA-bound
        warmup = const.tile([P, 1], F32, name="warmup")
        nc.scalar.activation(warmup[:], bq_sb[:, 0:1], AFT.Exp)

        # split k / v AllGathers: k completes first so phase-B scores start
        # while the v gather is still on the wire
        agk_in = dramp.tile([D * TL], BF16, name="agk_in")
        agk_out = dramp.tile([R * D * TL], BF16, addr_space="Shared",
                             name="agk_out")
        agv_in = dramp.tile([TL * VW], BF16, name="agv_in")
        agv_out = dramp.tile([R * TL * VW], BF16, addr_space="Shared",
                             name="agv_out")

        attnT = [persist.tile([P, TL], BF16, name=f"attnT{i}") for i in range(NDT)]
        w1sb = persist.tile([P, NDT, DFF], BF16, name="w1sb")
        w2sb = persist.tile([P, NFT, D], BF16, name="w2sb")

        NRC = NCH - NTT  # 28 remote chunks

        kv_scope = top.enter_context(ExitStack())
        kvp = kv_scope.enter_context(tc.tile_pool(name="kvp", bufs=1))
        qT = [kvp.tile([P, TL], BF16, name=f"qT{i}") for i in range(NDT)]
        kT_loc = [kvp.tile([P, TL], BF16, name=f"kTl{i}") for i in range(NDT)]
        v_loc = [kvp.tile([P, VW], BF16, name=f"vl{t}") for t in range(NTT)]
        vf = [kvp.tile([P, VW], BF16, name=f"vf{c}") for c in range(NRC)]
        # phase-A partial numerators+denominators (row 64), one per head
        locn = [[kvp.tile([HD + 1, TL], BF16, name=f"loc{p}_{s}")
                 for s in range(2)] for p in range(H // 2)]

        # ------------------------------------------------------------------
        # Phase 1: QKV projections straight from host-transposed x
        # ------------------------------------------------------------------
        with ExitStack() as ph1:
            xp = ph1.enter_context(tc.tile_pool(name="xp", bufs=1))
            psQ = ph1.enter_context(tc.tile_pool(name="psQ", bufs=2, space="PSUM"))
            psV = ph1.enter_context(tc.tile_pool(name="psV", bufs=2, space="PSUM"))

            xtb_sb = xp.tile([P, NDT, TL], BF16, name="xtb")
            nc.sync.dma_start(xtb_sb[:], xTb.ap())
            wk_sb = xp.tile([P, NDT, D], BF16, name="wk_sb")
            nc.sync.dma_start(wk_sb[:], wkc.ap())
            wv_sb = xp.tile([P, NDT, D], BF16, name="wv_sb")
            nc.scalar.dma_start(wv_sb[:], wv.ap())
            wq_sb = xp.tile([P, NDT, D], BF16, name="wq_sb")

            def proj_jt(w_sb, jt, bcol, out_ap):
                """qkv^T tile for channel block jt (0..5) of weight w_sb."""
                ps = psQ.tile([P, TL], F32, tag="psq", name="psq")
                for d_ in range(NDT):
                    nc.tensor.matmul(ps[:], w_sb[:, d_, P * jt:P * (jt + 1)],
                                     xtb_sb[:, d_, :],
                                     start=(d_ == 0), stop=(d_ == NDT - 1))
                nc.vector.tensor_scalar_add(out_ap, ps[:], bq_sb[:, bcol:bcol + 1])

            # k first (AllGather input): bias cols 6..11 of bq_sb.
            agk2 = agk_in.rearrange("(a b) -> a b", b=TL)
            for i in range(NDT):
                proj_jt(wk_sb, i, NDT + i, kT_loc[i][:])
                nc.sync.dma_start(agk2[P * i:P * (i + 1), :], kT_loc[i][:])

            # trigger the k gather as soon as k is staged; v follows on the
            # same cc stream so its wire time hides behind phase-A compute
            nc.gpsimd.collective_compute(
                "AllGather", mybir.AluOpType.bypass,
                replica_groups=[list(range(R))],
                ins=[agk_in[:]], outs=[agk_out[:]],
            )

            # v in [t, j] orientation with the per-head ones columns baked in
            agv2 = agv_in.rearrange("(a b) -> a b", b=VW)
            for tt in range(NTT):
                vfv = v_loc[tt][:].rearrange("p (h e) -> p h e", e=HD + 1)
                nc.vector.memset(vfv[:, :, HD:HD + 1], 1.0)
                for o2 in range(2):
                    sl = slice(384 * o2, 384 * (o2 + 1))
                    ps = psV.tile([P, 384], F32, tag="psv", name="psv")
                    for d_ in range(NDT):
                        nc.tensor.matmul(ps[:],
                                         xtb_sb[:, d_, P * tt:P * (tt + 1)],
                                         wv_sb[:, d_, sl],
                                         start=(d_ == 0), stop=False)
                    nc.tensor.matmul(ps[:], ones_row[:], bv_sb[:, sl],
                                     start=False, stop=True)
                    dst = vfv[:, 6 * o2:6 * (o2 + 1), 0:HD]
                    nc.vector.tensor_copy(
                        dst, ps[:].rearrange("p (h e) -> p h e", e=HD))
                nc.sync.dma_start(agv2[P * tt:P * (tt + 1), :], v_loc[tt][:])

            nc.gpsimd.collective_compute(
                "AllGather", mybir.AluOpType.bypass,
                replica_groups=[list(range(R))],
                ins=[agv_in[:]], outs=[agv_out[:]],
            )

            # q projections overlap with the collective (wq DMA deferred so
            # it never competes with the k/v path that gates the AllGather)
            nc.scalar.dma_start(wq_sb[:], wqc.ap())
            for i in range(NDT):
                proj_jt(wq_sb, i, i, qT[i][:])

        # Remote-rank readback views: rows of the gathered buffers
        agoK = agk_out.rearrange("(n w) -> n w", w=TL)   # [R*768, 512]
        agoV = agv_out.rearrange("(n w) -> n w", w=VW)   # [R*512, 780]

        ktp = kv_scope.enter_context(tc.tile_pool(name="ktp", bufs=2))

        def load_kt_remote(p_):
            """Indirect-gather the 7 remote ranks' kT d-tile p_ (row ids from
            the per-core kvidx table, so the local rank is skipped)."""
            kt = []
            for j in range(R - 1):
                t_ = ktp.tile([P, TL], BF16, tag=f"kt{j}", name=f"kt{j}")
                col = 7 * p_ + j
                nc.gpsimd.indirect_dma_start(
                    out=t_[:], out_offset=None,
                    in_=agoK[:],
                    in_offset=bass.IndirectOffsetOnAxis(
                        ap=kvidx_sb[:, col:col + 1], axis=0))
                kt.append(t_)
            return kt

        # ------------------------------------------------------------------
        # Phase 2: attention, head pairs (row-packed score matmuls)
        # ------------------------------------------------------------------
        with ExitStack() as ph2:
            scp = ph2.enter_context(tc.tile_pool(name="scp", bufs=2, space="PSUM"))
            accp = ph2.enter_context(tc.tile_pool(name="accp", bufs=3, space="PSUM"))
            bcp = ph2.enter_context(tc.tile_pool(name="bcp", bufs=1, space="PSUM"))
            weip = ph2.enter_context(tc.tile_pool(name="weip", bufs=6))
            tailp = ph2.enter_context(tc.tile_pool(name="tailp", bufs=2))

            def finish_pair(pend):
                """Normalize a finished pair: ONE merged reciprocal [2,TL]
                (halves the slow DVE recip cost) -> ONE sel2 broadcast matmul
                (K=2 picks rec row 0 for head-0 partitions, row 1 for head-1)
                -> multiply into attnT straight from PSUM. Emitted early in
                the NEXT pair so nothing lands on a pair boundary."""
                pp, num, den2 = pend
                rec2 = tailp.tile([33, TL], F32R, tag="rec2", name="rec2")
                with nc.allow_low_precision(reason="f32r recip, as baseline"):
                    nc.vector.reciprocal(rec2[:], den2[:].bitcast(F32R))
                bc = bcp.tile([P, TL], F32, tag="bc", name="bc")
                nc.tensor.matmul(bc[:], sel33[:, 0:P], rec2[:],
                                 start=True, stop=True)
                nc.vector.tensor_tensor(attnT[pp][:], num[:].bitcast(F32R),
                                        bc[:].bitcast(F32R),
                                        mybir.AluOpType.mult)

            def chunk_step(p_, kT_c, vtile, acc0, acc1, start, stop):
                h0, h1 = 2 * p_, 2 * p_ + 1
                sc = scp.tile([P, 2 * TL], F32, tag="sc", name="sc")
                nc.tensor.matmul(sc[:, 0:TL], kT_c[0:HD, :],
                                 qT[p_][0:HD, :], start=True, stop=True)
                nc.tensor.matmul(sc[:, TL:2 * TL], kT_c[HD:P, :],
                                 qT[p_][HD:P, :], start=True, stop=True)
                wei = weip.tile([P, 2 * TL], BF16, tag="wei", name="wei")
                nc.scalar.activation(wei[:], sc[:], AFT.Exp, scale=SCALE)
                nc.tensor.matmul(acc0[:],
                                 vtile[:, (HD + 1) * h0:(HD + 1) * (h0 + 1)],
                                 wei[:, 0:TL], start=start, stop=stop)
                nc.tensor.matmul(acc1[:],
                                 vtile[:, (HD + 1) * h1:(HD + 1) * (h1 + 1)],
                                 wei[:, TL:2 * TL], start=start, stop=stop)

            # ---- Phase A: local (diagonal) attention, fully overlapped
            # with the k/v AllGathers (needs no remote data) ----
            for p_ in range(H // 2):
                accA0 = accp.tile([HD + 1, TL], F32, tag="acc", name="accA0")
                accA1 = accp.tile([HD + 1, TL], F32, tag="acc", name="accA1")
                for s in range(NTT):
                    chunk_step(p_, kT_loc[p_][:, P * s:P * (s + 1)], v_loc[s][:],
                               accA0, accA1, s == 0, s == NTT - 1)
                nc.vector.tensor_copy(locn[p_][0][:], accA0[:])
                nc.vector.tensor_copy(locn[p_][1][:], accA1[:])

            # ---- Remote k/v readback (indirect gathers skip our own rank)
            kt0 = load_kt_remote(0)
            for c in range(NRC):
                nc.gpsimd.indirect_dma_start(
                    out=vf[c][:], out_offset=None, in_=agoV[:],
                    in_offset=bass.IndirectOffsetOnAxis(
                        ap=kvidx_sb[:, 7 * NDT + c:7 * NDT + c + 1], axis=0))

            # FFN weight prefetch on the sync ring, poisoned on the last vf
            # gather so it doesn't compete with the collectives for HBM
            nc.vector.tensor_copy(w1sb[0:1, 0, 0:1], vf[NRC - 1][0:1, 0:1])
            nc.sync.dma_start(w1sb[:], w1h.ap())
            nc.vector.tensor_copy(w2sb[0:1, 0, 0:1], vf[NRC - 1][0:1, 0:1])
            nc.sync.dma_start(w2sb[:], w2b.ap())

            # ---- Phase B: the 28 remote chunks per pair, accumulators
            # seeded with the phase-A partials via an identity matmul ----
            pend = None
            for p_ in range(H // 2):
                kt = kt0 if p_ == 0 else load_kt_remote(p_)
                acc0 = accp.tile([HD + 1, TL], F32, tag="acc", name="acc0")
                acc1 = accp.tile([HD + 1, TL], F32, tag="acc", name="acc1")
                nc.tensor.matmul(acc0[:], i65[:, 0:HD + 1], locn[p_][0][:],
                                 start=True, stop=False)
                nc.tensor.matmul(acc1[:], i65[:, 0:HD + 1], locn[p_][1][:],
                                 start=True, stop=False)
                for c in range(NRC):
                    j, s = c // NTT, c % NTT
                    chunk_step(p_, kt[j][:, P * s:P * (s + 1)], vf[c][:],
                               acc0, acc1, False, c == NRC - 1)
                    if c == 8 and pend is not None:
                        finish_pair(pend)
                        pend = None
                # Evacuate numerators + denominators to SBUF right away so
                # the acc PSUM banks free fast; the slow reciprocal and the
                # broadcast run later, off the critical path.
                num = tailp.tile([P, TL], F32, tag="num", name="num")
                nc.vector.tensor_copy(num[0:HD, :], acc0[0:HD, :])
                nc.vector.tensor_copy(num[HD:P, :], acc1[0:HD, :])
                den2 = tailp.tile([33, TL], F32, tag="den2", name="den2")
                nc.vector.memset(den2[:], 1.0)
                nc.vector.tensor_copy(den2[0:1, :], acc0[HD:HD + 1, :])
                nc.vector.tensor_copy(den2[32:33, :], acc1[HD:HD + 1, :])
                pend = (p_, num, den2)
            finish_pair(pend)

        kv_scope.close()

        # ------------------------------------------------------------------
        # Phase 3: FFN1 (gelu) pipelined with FFN2 sweep A, then sweep B
        # ------------------------------------------------------------------
        hTp = top.enter_context(tc.tile_pool(name="hTp", bufs=1))
        hT = [hTp.tile([P, TL], BF16, name=f"hT{f}") for f in range(NFT)]

        with ExitStack() as ph3:
            ps1 = ph3.enter_context(tc.tile_pool(name="ps1", bufs=2, space="PSUM"))
            ps2 = ph3.enter_context(tc.tile_pool(name="ps2", bufs=1, space="PSUM"))
            outp = ph3.enter_context(tc.tile_pool(name="outp", bufs=1))
            out_sb = [outp.tile([P, D], F32, name=f"out{tt}") for tt in range(NTT)]

            acc2 = {}
            for tt in (0, 1):
                for o2 in range(2):
                    acc2[(tt, o2)] = ps2.tile([P, 384], F32, tag=f"g{tt}{o2}",
                                              name=f"acc2_{tt}_{o2}")
            for ft in range(NFT):
                ps = ps1.tile([P, TL], F32, tag="ps1t", name="ps1t")
                for d_ in range(NDT):
                    nc.tensor.matmul(ps[:], w1sb[:, d_, P * ft:P * (ft + 1)],
                                     attnT[d_][:],
                                     start=(d_ == 0), stop=(d_ == NDT - 1))
                nc.scalar.activation(hT[ft][:], ps[:], AFT.Gelu,
                                     bias=b1_sb[:, ft:ft + 1])
                # FFN2 sweep A accumulates as soon as each hT tile is ready
                for tt in (0, 1):
                    for o2 in range(2):
                        nc.tensor.matmul(acc2[(tt, o2)][:],
                                         hT[ft][:, P * tt:P * (tt + 1)],
                                         w2sb[:, ft, 384 * o2:384 * (o2 + 1)],
                                         start=(ft == 0), stop=False)
            for tt in (0, 1):
                for o2 in range(2):
                    sl = slice(384 * o2, 384 * (o2 + 1))
                    nc.tensor.matmul(acc2[(tt, o2)][:], ones_row[:], b2_sb[:, sl],
                                     start=False, stop=True)
                    nc.vector.tensor_copy(out_sb[tt][:, sl], acc2[(tt, o2)][:])
                nc.sync.dma_start(y.ap()[P * tt:P * (tt + 1), :], out_sb[tt][:])

            # sweep B (reuses the same 4 PSUM banks after sweep A evacuates)
            accB = {}
            for tt in (2, 3):
                for o2 in range(2):
                    accB[(tt, o2)] = ps2.tile([P, 384], F32, tag=f"g{tt - 2}{o2}",
                                              name=f"acc2_{tt}_{o2}")
            for ft in range(NFT):
                for tt in (2, 3):
                    for o2 in range(2):
                        nc.tensor.matmul(accB[(tt, o2)][:],
                                         hT[ft][:, P * tt:P * (tt + 1)],
                                         w2sb[:, ft, 384 * o2:384 * (o2 + 1)],
                                         start=(ft == 0), stop=False)
            for tt in (2, 3):
                for o2 in range(2):
                    sl = slice(384 * o2, 384 * (o2 + 1))
                    nc.tensor.matmul(accB[(tt, o2)][:], ones_row[:], b2_sb[:, sl],
                                     start=False, stop=True)
                    nc.vector.tensor_copy(out_sb[tt][:, sl], accB[(tt, o2)][:])
                nc.sync.dma_start(y.ap()[P * tt:P * (tt + 1), :], out_sb[tt][:])

    return nc


def _get_nc():
    if "nc" not in _NC_CACHE:
        _NC_CACHE["nc"] = _build_nc()
    return _NC_CACHE["nc"]


def _prep_common(inputs):
    w_qkv = np.ascontiguousarray(np.asarray(inputs["w_qkv"], np.float32))
    w_ff1 = np.ascontiguousarray(np.asarray(inputs["w_ff1"], np.float32))
    common = {
        # q columns, bf16, d-tile-major: [128, 6, 768]
        "wqc": np.ascontiguousarray(
            w_qkv[:, 0:D].reshape(NDT, P, D).transpose(1, 0, 2)
        ).astype(ml_dtypes.bfloat16),
        # k columns, bf16, d-tile-major: [128, 6, 768]
        "wkc": np.ascontiguousarray(
            w_qkv[:, D:2 * D].reshape(NDT, P, D).transpose(1, 0, 2)
        ).astype(ml_dtypes.bfloat16),
        # v columns, bf16, d-tile-major: [128, 6, 768]
        "wv": np.ascontiguousarray(
            w_qkv[:, 2 * D:].reshape(NDT, P, D).transpose(1, 0, 2)
        ).astype(ml_dtypes.bfloat16),
        # w_ff1 d-tile-major, bf16: [128, 6, 3072]
        "w1h": np.ascontiguousarray(
            w_ff1.reshape(NDT, P, DFF).transpose(1, 0, 2)
        ).astype(ml_dtypes.bfloat16),
        # w_ff2 ff-tile-major, bf16: [128, 24, 768]
        "w2b": np.ascontiguousarray(
            np.asarray(inputs["w_ff2"], np.float32)
            .reshape(NFT, P, D).transpose(1, 0, 2)).astype(ml_dtypes.bfloat16),
        "b_qkv": np.ascontiguousarray(np.asarray(inputs["b_qkv"], np.float32)),
        "b_ff1": np.ascontiguousarray(np.asarray(inputs["b_ff1"], np.float32)),
        "b_ff2": np.ascontiguousarray(np.asarray(inputs["b_ff2"], np.float32)),
    }
    return common


def run_sharded(inputs, **run_kwargs):
    """Run the SPMD kernel; returns (full_output [1,4096,768], BassKernelResults)."""
    x = np.ascontiguousarray(np.asarray(inputs["x"], dtype=np.float32))
    assert x.shape == (1, T, D), x.shape
    common = _prep_common(inputs)
    in_maps = []
    rng = np.arange(P, dtype=np.int32)
    for r in range(R):
        m = dict(common)
        xr = x[0, TL * r:TL * (r + 1), :]  # [512, 768]
        xT = np.ascontiguousarray(xr.T.reshape(NDT, P, TL).transpose(1, 0, 2))
        m["xTb"] = xT.astype(ml_dtypes.bfloat16)
        # remote-rank row indices into the gathered k ([R*768, 512] rows)
        # and v ([R*512, 780] rows) buffers; local rank excluded
        perm = [(r + 1 + i) % R for i in range(R - 1)]
        kvi = np.empty((P, 7 * NDT + 28), np.int32)
        for p_ in range(NDT):
            for j in range(R - 1):
                kvi[:, 7 * p_ + j] = perm[j] * D + P * p_ + rng
        for c in range(28):
            kvi[:, 7 * NDT + c] = perm[c // NTT] * TL + P * (c % NTT) + rng
        m["kvidx"] = kvi
        in_maps.append(m)
    nc = _get_nc()
    res = run_bass_kernel_spmd(nc, in_maps, core_ids=list(range(R)), **run_kwargs)
    out = np.concatenate([res.results[r]["y"] for r in range(R)], axis=0)
    return out.reshape(1, T, D), res


def kernel(**inputs):
    out, _ = run_sharded(inputs)
    return out

